# revision 39
# baseline (speedup 1.0000x reference)
"""Trainium2 Bass kernel for nn_PhysicsMessageGNN (3-step GNN message passing).

Sharding: row-wise across 8 NeuronCores. Core r is FED rows
[r*512, (r+1)*512) of adj/dist (and the matching x rows) — the program itself
is rank-free (identical SPMD). Core r computes its slice of
m = (pot*adj) @ h and the h-update for its own rows; h is replicated via an
ncfw AllGather between steps.

On-chip layouts are "transposed" [j-or-feature on partitions, i on free] so
every matmul contracts over partitions with no per-step transposes:
  hT      [128 d, 4096 j]   f32 global h, feature-major (from AllGather)
  hT_own  [128 d, 512 i]    f32 this core's own rows (local update pipeline)
  h_nat   [j, d] blocks     f32 + bf16 copies (PE-transposed from hT per step)
  A1T     [j, i_own] bf16   (exp(-r/3)*adj).T slice  — via masked-r trick
  A2T     [j, i_own] f32    (0.04*(sr6^2-sr6)*adj).T slice
Masked-r trick: rt = dist - 100*adj; Ln/Exp consume it with a +100-shifted
bias so adj==1 entries see exactly r and adj==0 entries see r+100 (decay/LJ
~1e-15 ≈ 0). This kills all explicit adj multiplies.
Per step (jc = 32 chunks of 128 j):
  psT_jc = hT[:,jc].T @ hT_own                      (PE, f32r, 1 MM)
  W_jc   = max(psT_jc,0) * A1T[:,jc]                (DVE scalar_tensor_tensor)
  mT    += h_natb[jc].T @ W_jc + h_natf[jc].T @ A2T[:,jc]   (PE bf16 + f32r)
then MLP + residual + partition-axis layernorm on [128, 512] tiles, and an
AllGather of hT_own into hT (steps 0,1 only).
"""

import math
import numpy as np

N = 4096
F = 64
H = 128
NCORES = 8
ROWS = N // NCORES  # 512
NJC = N // H        # 32
SCALE = 3.0
LN_EPS = 1e-5
MASK_OFF = 100.0
B6 = 6.0 * math.log(3.5) + math.log(0.04)

_CACHE = {}


def _build_program(single_core=False):
    import concourse.tile as tile
    from concourse import bacc, mybir
    from concourse.masks import make_identity
    from contextlib import ExitStack

    dt = mybir.dt
    Alu = mybir.AluOpType
    Act = mybir.ActivationFunctionType
    f32r = dt.float32r

    import concourse.bacc as _bacc_mod
    from concourse import hw_specs as _hw_specs
    _orig_tables = _hw_specs.get_activation_tables

    def _patched_tables(arch):
        t = _orig_tables(arch)
        for name, fns in t.items():
            if name != "natural_log_exp_and_others":
                fns.discard(Act.Exp)
                fns.discard(Act.Ln)
        return t

    _bacc_mod.get_activation_tables = _patched_tables

    nc = bacc.Bacc("TRN2", target_bir_lowering=False, debug=False,
                   num_devices=1 if single_core else NCORES)

    x_d = nc.dram_tensor("x", [N, F], dt.float32, kind="ExternalInput").ap()
    xo_d = nc.dram_tensor("x_own", [ROWS, F], dt.float32, kind="ExternalInput").ap()
    rtT_d = nc.dram_tensor("rtT_s", [N, ROWS], dt.float32, kind="ExternalInput").ap()
    Win_d = nc.dram_tensor("W_in", [F, H], dt.float32, kind="ExternalInput").ap()
    bin_d = nc.dram_tensor("b_in", [H, 1], dt.float32, kind="ExternalInput").ap()
    W1a_d = nc.dram_tensor("W1a", [H, H], dt.float32, kind="ExternalInput").ap()
    W1b_d = nc.dram_tensor("W1b", [H, H], dt.float32, kind="ExternalInput").ap()
    b1_d = nc.dram_tensor("b_u1", [H, 1], dt.float32, kind="ExternalInput").ap()
    W2_d = nc.dram_tensor("W_u2", [H, H], dt.float32, kind="ExternalInput").ap()
    b2_d = nc.dram_tensor("b_u2", [H, 1], dt.float32, kind="ExternalInput").ap()
    gam_d = nc.dram_tensor("gamma", [H, 1], dt.float32, kind="ExternalInput").ap()
    bet_d = nc.dram_tensor("beta", [H, 1], dt.float32, kind="ExternalInput").ap()
    out_d = nc.dram_tensor("hT_out", [H, ROWS], dt.float32, kind="ExternalOutput").ap()

    with tile.TileContext(nc) as tc:
        ctx = ExitStack()
        const_pool = ctx.enter_context(tc.tile_pool(name="const", bufs=1))
        persist = ctx.enter_context(tc.tile_pool(name="persist", bufs=1))
        dram = ctx.enter_context(tc.tile_pool(name="dram", bufs=1, space="DRAM"))

        id_f32 = const_pool.tile([128, 128], dt.float32)
        make_identity(nc, id_f32[:])
        id_bf = const_pool.tile([128, 128], dt.bfloat16)
        make_identity(nc, id_bf[:])
        ones_128th_f = const_pool.tile([128, 1], dt.float32)
        nc.gpsimd.memset(ones_128th_f[:], 1.0 / 128.0)
        ones_128th = const_pool.tile([128, 1], dt.float32r)
        nc.scalar.activation(ones_128th[:], ones_128th_f[:], Act.Copy)
        ones_row_f = const_pool.tile([1, 128], dt.float32)
        nc.gpsimd.memset(ones_row_f[:], 1.0)
        ones_row = const_pool.tile([1, 128], dt.float32r)
        nc.scalar.activation(ones_row[:], ones_row_f[:], Act.Copy)
        id_f32r = const_pool.tile([128, 128], dt.float32r)
        nc.scalar.activation(id_f32r[:], id_f32[:], Act.Copy)

        def const_scalar(name, val):
            t = const_pool.tile([128, 1], dt.float32, name=name)
            nc.gpsimd.memset(t[:], val)
            return t

        c_decay_bias = const_scalar("c_decay_bias", -MASK_OFF / SCALE)
        c_maskoff = const_scalar("c_maskoff", MASK_OFF)
        c_b6 = const_scalar("c_b6", B6)
        c_lneps = const_scalar("c_lneps", LN_EPS)

        def load_const(name, dram_ap, shape, dtype=dt.float32):
            t = const_pool.tile(shape, dtype, name=name)
            src_ap = dram_ap[:]
            if dtype == dt.float32r:
                src_ap = src_ap.bitcast(dt.float32r)
            nc.sync.dma_start(t[:], src_ap)
            return t

        Win_sb = load_const("Win_sb", Win_d, [F, H], dt.float32r)
        bin_sb = load_const("bin_sb", bin_d, [H, 1])
        W1a_sb = load_const("W1a_sb", W1a_d, [H, H], dt.float32r)
        W1b_sb = load_const("W1b_sb", W1b_d, [H, H], dt.float32r)
        b1_sb = load_const("b1_sb", b1_d, [H, 1])
        W2_sb = load_const("W2_sb", W2_d, [H, H], dt.float32r)
        b2_sb = load_const("b2_sb", b2_d, [H, 1])
        gam_sb = load_const("gam_sb", gam_d, [H, 1])
        bet_sb = load_const("bet_sb", bet_d, [H, 1])

        hT = persist.tile([H, N], dt.float32r)
        hT_own = persist.tile([H, ROWS], dt.float32r)
        h_natf = persist.tile([128, NJC * H], dt.float32r)
        A1T = persist.tile([128, NJC * ROWS], dt.bfloat16)
        A2T = persist.tile([128, NJC * ROWS], dt.float32r)

        # ================= h init ============================================
        with (
            tc.tile_pool(name="xinit", bufs=3) as xpool,
            tc.tile_pool(name="xinit_ps", bufs=2, space="PSUM") as xps,
            tc.tile_pool(name="hinit_ps", bufs=2, space="PSUM") as hps,
        ):
            xT = xpool.tile([F, N], dt.float32r, tag="xT")
            for c in range(8):
                ps_xt = xps.tile([F, 512], dt.float32, tag="xt")
                for k in range(4):
                    xch = xpool.tile([128, F], dt.float32, tag="xch")
                    i0 = (4 * c + k) * 128
                    nc.sync.dma_start(xch[:], x_d[i0:i0 + 128, :])
                    nc.tensor.transpose(ps_xt[:, k * 128:(k + 1) * 128], xch[:], id_f32[:])
                nc.scalar.activation(xT[:, c * 512:(c + 1) * 512], ps_xt[:], Act.Copy)
            for c in range(8):
                ps_h = hps.tile([H, 512], dt.float32, tag="h")
                nc.tensor.matmul(ps_h[:], Win_sb[:],
                                 xT[:, c * 512:(c + 1) * 512],
                                 start=True, stop=True)
                nc.vector.tensor_scalar_add(hT[:, c * 512:(c + 1) * 512], ps_h[:], bin_sb[:])
            # own rows -> hT_own
            xoT = xpool.tile([F, ROWS], dt.float32r, tag="xoT")
            ps_xo = xps.tile([F, 512], dt.float32, tag="xt")
            for k in range(4):
                xch = xpool.tile([128, F], dt.float32, tag="xch")
                nc.sync.dma_start(xch[:], xo_d[k * 128:(k + 1) * 128, :])
                nc.tensor.transpose(ps_xo[:, k * 128:(k + 1) * 128], xch[:], id_f32[:])
            nc.scalar.activation(xoT[:], ps_xo[:], Act.Copy)
            ps_ho = hps.tile([H, 512], dt.float32, tag="h")
            nc.tensor.matmul(ps_ho[:], Win_sb[:], xoT[:],
                             start=True, stop=True)
            nc.vector.tensor_scalar_add(hT_own[:], ps_ho[:], bin_sb[:])

        # ================= initial h_nat build ===============================
        with tc.tile_pool(name="hnat_psi", bufs=2, space="PSUM") as nps:
            for g in range(8):
                ps_n = nps.tile([128, 512], dt.float32r, tag="n")
                for k in range(4):
                    jc = 4 * g + k
                    nc.tensor.transpose(ps_n[:, k * 128:(k + 1) * 128],
                                        hT[:, jc * 128:(jc + 1) * 128], id_f32r[:])
                nc.vector.tensor_copy(h_natf[:, g * 512:(g + 1) * 512], ps_n[:])

        # ================= A-matrix prep (direct transposed layout) ==========
        # rtT arrives already [j, i_own]; compute A1T/A2T in place, no PE
        # transposes, no PSUM evacuations. Units cover 2 j-chunks (1024 cols).
        rtT_v = rtT_d.rearrange("(k p) i -> p k i", p=128)
        with (
            tc.tile_pool(name="prep_in", bufs=4) as pin,
            tc.tile_pool(name="prep_t", bufs=3) as ptmp,
        ):
            for u in range(NJC // 2):
                rtt = pin.tile([128, 2 * ROWS], dt.float32, tag="din")
                nc.sync.dma_start(
                    rtt[:].rearrange("p (k i) -> p k i", k=2),
                    rtT_v[:, 2 * u:2 * u + 2, :])
                o = slice(u * 2 * ROWS, (u + 1) * 2 * ROWS)
                # A1T = exp(-(rt+100)/3) directly into the persistent tile
                nc.scalar.activation(A1T[:, o], rtt[:], Act.Exp,
                                     bias=c_decay_bias[:], scale=-1.0 / SCALE)
                L = ptmp.tile([128, 2 * ROWS], dt.float32, tag="L")
                nc.scalar.activation(L[:], rtt[:], Act.Ln, bias=c_maskoff[:], scale=1.0)
                xp = ptmp.tile([128, 2 * ROWS], dt.float32, tag="xp")
                nc.scalar.activation(xp[:], L[:], Act.Exp, bias=c_b6[:], scale=-6.0)
                xpsq = ptmp.tile([128, 2 * ROWS], dt.float32, tag="xpsq")
                nc.gpsimd.tensor_tensor(xpsq[:], xp[:], xp[:], op=Alu.mult)
                nc.vector.scalar_tensor_tensor(A2T[:, o], xpsq[:], 25.0, xp[:],
                                               op0=Alu.mult, op1=Alu.subtract)

        # ================= AG bounce buffers =================================
        ag_in = [dram.tile([2 * H, ROWS], dt.float32r, name=f"ag_in{s}") for s in range(2)]
        if not single_core:
            ag_out = [dram.tile([2 * H * NCORES, ROWS], dt.float32r, addr_space="Shared",
                                name=f"ag_out{s}") for s in range(2)]

        # ================= steps =============================================
        step_pool = ctx.enter_context(tc.tile_pool(name="step", bufs=5))
        mlp_pool = ctx.enter_context(tc.tile_pool(name="mlp", bufs=2))
        ps_pool = ctx.enter_context(tc.tile_pool(name="ps_ps", bufs=3, space="PSUM"))
        mt_pool = ctx.enter_context(tc.tile_pool(name="mt_ps", bufs=1, space="PSUM"))

        for step in range(3):
            mT = mt_pool.tile([H, ROWS], dt.float32, tag="mt")
            for jc in range(NJC):
                ps_t = ps_pool.tile([128, ROWS], dt.float32, tag="pst")
                nc.tensor.matmul(ps_t[:], hT[:, jc * 128:(jc + 1) * 128],
                                 hT_own[:], start=True, stop=True)
                W_jc = step_pool.tile([128, ROWS], dt.float32r, tag="wjc")
                nc.vector.scalar_tensor_tensor(W_jc[:], ps_t[:], 0.0,
                                               A1T[:, jc * ROWS:(jc + 1) * ROWS],
                                               op0=Alu.max, op1=Alu.mult)
                nc.tensor.matmul(mT[:], h_natf[:, jc * 128:(jc + 1) * 128], W_jc[:],
                                 start=(jc == 0), stop=False)
                nc.tensor.matmul(mT[:], h_natf[:, jc * 128:(jc + 1) * 128],
                                 A2T[:, jc * ROWS:(jc + 1) * ROWS],
                                 start=False, stop=(jc == NJC - 1))
            # ---- MLP: delta = relu([h|m] @ W_u1 + b1) @ W_u2 + b2 ----
            misc_ps = ctx0 = tc.tile_pool(name=f"misc_ps{step}", bufs=1, space="PSUM")
            misc_ps = misc_ps.__enter__()
            mT_sb = mlp_pool.tile([H, ROWS], dt.float32r, tag="mtsb")
            nc.scalar.activation(mT_sb[:], mT[:], Act.Copy)
            d1 = misc_ps.tile([H, ROWS], dt.float32, tag="mlp")
            nc.tensor.matmul(d1[:], W1a_sb[:], hT_own[:],
                             start=True, stop=False)
            nc.tensor.matmul(d1[:], W1b_sb[:], mT_sb[:],
                             start=False, stop=True)
            r1 = mlp_pool.tile([H, ROWS], dt.float32r, tag="r1")
            nc.scalar.activation(r1[:], d1[:], Act.Relu, bias=b1_sb[:])
            d2 = misc_ps.tile([H, ROWS], dt.float32, tag="mlp")
            nc.tensor.matmul(d2[:], W2_sb[:], r1[:],
                             start=True, stop=True)
            # t = hT_own + d2 + b2
            t_sb = mlp_pool.tile([H, ROWS], dt.float32r, tag="t")
            nc.vector.scalar_tensor_tensor(t_sb[:], d2[:], b2_sb[:],
                                           hT_own[:].bitcast(dt.float32),
                                           op0=Alu.add, op1=Alu.add)
            # ---- layernorm over partitions via ones-matmuls ----
            tsq = mlp_pool.tile([H, ROWS], dt.float32r, tag="tsq")
            nc.scalar.activation(tsq[:], t_sb[:].bitcast(dt.float32), Act.Square)
            sums_ps = misc_ps.tile([1, 2 * ROWS], dt.float32, tag="sums")
            s0_ps, s1_ps = sums_ps[:, 0:ROWS], sums_ps[:, ROWS:2 * ROWS]
            nc.tensor.matmul(s0_ps, ones_128th[:],
                             t_sb[:], start=True, stop=True)
            nc.tensor.matmul(s1_ps, ones_128th[:],
                             tsq[:], start=True, stop=True)
            # var = E[x^2] - mean^2 ;  rstd = exp(-0.5*ln(var+eps))
            msq = mlp_pool.tile([1, ROWS], dt.float32, tag="msq")
            nc.scalar.activation(msq[:], s0_ps, Act.Square)
            var_sb = mlp_pool.tile([1, ROWS], dt.float32, tag="var")
            nc.vector.scalar_tensor_tensor(var_sb[:], s1_ps, 1.0,
                                           msq[:], op0=Alu.mult, op1=Alu.subtract)
            lnv = mlp_pool.tile([1, ROWS], dt.float32, tag="lnv")
            nc.scalar.activation(lnv[:], var_sb[:], Act.Ln, bias=c_lneps[0:1, :])
            rstd = mlp_pool.tile([1, ROWS], dt.float32r, tag="rstd")
            nc.scalar.activation(rstd[:], lnv[:], Act.Exp, scale=-0.5)
            # nmr = -mean * rstd
            nmr = mlp_pool.tile([1, ROWS], dt.float32r, tag="nmr")
            nc.vector.scalar_tensor_tensor(nmr[:], s0_ps, -1.0,
                                           rstd[:].bitcast(dt.float32),
                                           op0=Alu.mult, op1=Alu.mult)
            # broadcast rstd / nmr to 128 partitions (shared bank, sequential)
            bc0 = misc_ps.tile([H, ROWS], dt.float32, tag="bc")
            nc.tensor.matmul(bc0[:], ones_row[:], rstd[:], start=True, stop=True)
            u_sb = mlp_pool.tile([H, ROWS], dt.float32, tag="u")
            nc.vector.scalar_tensor_tensor(u_sb[:], bc0[:], 1.0,
                                           t_sb[:].bitcast(dt.float32),
                                           op0=Alu.mult, op1=Alu.mult)
            bc1 = misc_ps.tile([H, ROWS], dt.float32, tag="bc")
            nc.tensor.matmul(bc1[:], ones_row[:], nmr[:], start=True, stop=True)
            v_sb = mlp_pool.tile([H, ROWS], dt.float32, tag="v")
            nc.vector.scalar_tensor_tensor(v_sb[:], bc1[:], 1.0, u_sb[:],
                                           op0=Alu.mult, op1=Alu.add)
            # gamma/beta -> new hT_own
            nc.vector.tensor_scalar(hT_own[:], v_sb[:], gam_sb[:], bet_sb[:],
                                    op0=Alu.mult, op1=Alu.add)
            ctx0.__exit__(None, None, None)

            if step < 2:
                # own rows, natural layout, via 4 PE transposes
                with tc.tile_pool(name=f"hop{step}", bufs=1, space="PSUM") as hop:
                    ps_o = hop.tile([128, 512], dt.float32r, tag="o")
                    for k in range(4):
                        nc.tensor.transpose(ps_o[:, k * 128:(k + 1) * 128],
                                            hT_own[:, k * 128:(k + 1) * 128], id_f32r[:])
                    ho_nat = step_pool.tile([128, 512], dt.float32r, tag="honat")
                    nc.vector.tensor_copy(ho_nat[:], ps_o[:])
                # packed payload: rows 0:128 = hT_own; rows 128:256 = h_own natural
                nc.sync.dma_start(ag_in[step][0:H, :], hT_own[:])
                nat_half = ag_in[step][H:2 * H, :].rearrange("x c -> (x c)").rearrange(
                    "(k p c) -> p k c", k=4, p=128)
                nc.sync.dma_start(nat_half, ho_nat[:].rearrange("p (k c) -> p k c", c=128))
                if single_core:
                    for r in range(NCORES):
                        nc.sync.dma_start(hT[:, r * ROWS:(r + 1) * ROWS], ag_in[step][0:H, :])
                        nc.sync.dma_start(
                            h_natf[:, r * 4 * H:(r + 1) * 4 * H].rearrange(
                                "p (k c) -> p k c", c=128),
                            ag_in[step][H:2 * H, :].rearrange("x c -> (x c)").rearrange(
                                "(k p c) -> p k c", k=4, p=128))
                else:
                    nc.gpsimd.collective_compute(
                        "AllGather", Alu.bypass,
                        replica_groups=[list(range(NCORES))],
                        ins=[ag_in[step][:].opt()],
                        outs=[ag_out[step][:].opt()],
                    )
                    ago = ag_out[step]
                    # hT: rank r block rows [2H*r : 2H*r + H]; split for earlier unblock
                    hts = ago.rearrange("(r q p) f -> q p r f", r=NCORES, q=2)[0]
                    nc.sync.dma_start(
                        hT[:, 0:N // 2].rearrange("p (r f) -> p r f", r=NCORES // 2),
                        hts[:, 0:NCORES // 2, :])
                    nc.sync.dma_start(
                        hT[:, N // 2:N].rearrange("p (r f) -> p r f", r=NCORES // 2),
                        hts[:, NCORES // 2:NCORES, :])
                    # h_nat: rank r rows [2H*r + H : 2H*(r+1)] viewed [4k x 128p, 128c]
                    nat_src = ago.rearrange("(r q x) c -> q r (x c)", r=NCORES, q=2)[1]
                    for r in range(NCORES):
                        nc.sync.dma_start(
                            h_natf[:, r * 4 * H:(r + 1) * 4 * H].rearrange(
                                "p (k c) -> p k c", c=128),
                            nat_src[r].rearrange("(k p c) -> p k c", k=4, p=128))
            else:
                nc.sync.dma_start(out_d[:].bitcast(dt.float32r), hT_own[:])

        ctx.close()

    nc.compile()
    return nc


def _get_program():
    if "nc" not in _CACHE:
        _CACHE["nc"] = _build_program()
    return _CACHE["nc"]


def make_in_maps(inputs):
    x = np.ascontiguousarray(inputs["x"], np.float32)
    adj = np.asarray(inputs["adj"], np.float32)
    dist = np.asarray(inputs["dist_mat"], np.float32)
    # fused masked distance (pure f32 input transform; same values the device
    # STT produced): adj==1 -> dist-100, adj==0 -> dist; device biases by +100
    rt_full = dist - np.float32(100.0) * adj
    W_u1 = np.ascontiguousarray(inputs["W_u1"], np.float32)
    col = lambda v: np.ascontiguousarray(np.asarray(v, np.float32).reshape(H, 1))
    common = {
        "x": x,
        "W_in": np.ascontiguousarray(inputs["W_in"], np.float32),
        "b_in": col(inputs["b_in"]),
        "W1a": np.ascontiguousarray(W_u1[:H, :]),
        "W1b": np.ascontiguousarray(W_u1[H:, :]),
        "b_u1": col(inputs["b_u1"]),
        "W_u2": np.ascontiguousarray(inputs["W_u2"], np.float32),
        "b_u2": col(inputs["b_u2"]),
        "gamma": col(inputs["gamma"]),
        "beta": col(inputs["beta"]),
    }
    in_maps = []
    for r in range(NCORES):
        sl = slice(r * ROWS, (r + 1) * ROWS)
        in_maps.append({
            **common,
            "x_own": np.ascontiguousarray(x[sl]),
            "rtT_s": np.ascontiguousarray(rt_full[sl].T),
        })
    return in_maps


def kernel(**inputs):
    from concourse.bass_utils import run_bass_kernel_spmd
    nc = _get_program()
    in_maps = make_in_maps(inputs)
    res = run_bass_kernel_spmd(nc, in_maps, core_ids=list(range(NCORES)))
    out = np.concatenate(
        [res.results[r]["hT_out"].T for r in range(NCORES)], axis=0)
    return np.ascontiguousarray(out.astype(np.float32))
